# revision 14
# baseline (speedup 1.0000x reference)
"""Trainium2 Bass kernel v2 for nn_CrispToFuzzyConv (hypergraph message passing).

Math (see reference):
  Xe   = segment_sum(X[vertex], edges, E)                 # round 1
  Xv   = concat([deg * X, Xv2]),  Xv2 = segment_sum(Xe[edges], vertex, N)
  center = Xv @ w_b + b_b
  HL = center - (|Xv| @ w_a + b_a)
  HR = center + (|Xv| @ w_c + b_c)

v2 strategy (vs v1 which used dma_scatter_add):
  - NO scatter-adds.  Segment sums are computed as one-hot matmuls on the
    PE: gathered token rows [128tok x F] are contracted against on-chip
    one-hot matrices built with one DVE op each:
        onehot = ((iota - segid) == -(window_base))    # tensor_scalar
    For stage A the PSUM window is Xe^T [F x 512 edge-cols]; for stage C
    it is Xv2^T [F x 512 node-cols] which feeds the dense head directly
    as matmul lhsT (no transposes in the dense path; the host supplies
    deg-scaled X^T).
  - Everything is bf16 on the wire (gather rows are 256B packets, the
    AllGather payload halves); PSUM accumulates in f32.
  - SPMD: one program for all 8 cores.  The schedule is static, sized by
    the max token count over cores per (window, chunk/region) cell; each
    cell is one 1024-slot gather call whose runtime count (num_idxs_reg)
    is that max, so padding costs ~3% packets, not 30%.
  - Stage C accumulates region passes into an SBUF-resident Xv2 table
    (24.5KB/partition) so PSUM windows stay short-lived; the dense head
    runs fused per 128-node group right after its window closes.
  - Two AllGathers (edge regions) pipeline behind stage A / stage C
    descriptor generation.

Hardware constraints baked in (from v1 + this session's trace analysis):
  - dma_gather idx dtype is int16 -> gather tables <= 32767 rows: X is
    split in 4 chunks of 25000, Xe tables in 2 regions (24576/25600 rows)
  - elem_size_bytes % 256 == 0 -> bf16 F=128 rows (256B) are legal
  - gather output layout: token i -> partition i%128, col-block i//128,
    which is exactly the PE contraction layout
  - SWDGE data packets ride all 16 DMA engines at ~45ns/packet: the
    bottleneck is PACKET COUNT (= tokens), so padding is minimized and
    all other traffic is kept off the critical path
  - stale SBUF slots beyond a call's runtime count are made safe by
    memsetting the gather pool buffers once at startup (0.0 * onehot 0
    contributes nothing; avoids NaN poisoning)
"""

import numpy as np

# ---------------------------------------------------------------- constants
N = 100000
E = 50000
NNZ = 300000
F = 128
NC = 8

NODE_SH = 12500              # nodes per core
NODE_SH_P = 12544            # 98 groups of 128
NGRP = 98
CW = 25                      # 512-node windows (last = 256)

EDGE_SH = 6250               # edges per core
EDGE_SH_P = 6272             # 49 subwindows of 128
AW = 13                      # 512-edge windows (last = 128)
NCH = 4
CHUNK = 25000                # X chunk rows (int16 gather limit)

REG_SPLIT = 3072             # local edge rows [0,3072) -> region 0
REG_ROWS = (3072, 3200)      # padded local rows per region
TBL_ROWS = (NC * 3072, NC * 3200)

T = 1024                     # slots per gather call (8 blocks of 128)

_STATE = {}


# ---------------------------------------------------------------- host side
def _wrap16(idx):
    """[ncall, 1024] int -> [128, ncall*64] int16 SBUF image (idx i at
    partition i%16, col i//16; replicated across the 8 groups of 16)."""
    ncall = idx.shape[0]
    t = idx.reshape(ncall, 64, 16).transpose(0, 2, 1).astype(np.int16)
    t = np.tile(t, (1, 8, 1))                      # [ncall, 128, 64]
    return np.ascontiguousarray(t.transpose(1, 0, 2).reshape(128, ncall * 64))


def _seg_img(seg):
    """[ncall, 1024] f32 -> [128, ncall*8] f32 SBUF image (slot b*128+p of
    call k -> partition p, col k*8+b)."""
    ncall = seg.shape[0]
    t = seg.reshape(ncall, 8, 128).transpose(2, 0, 1).astype(np.float32)
    return np.ascontiguousarray(t.reshape(128, ncall * 8))


def _build_stream(owner, cell, gidx, segid, ncells):
    """Pack tokens into per-(core, cell) sorted slot arrays."""
    ga = np.zeros((NC, ncells, T), np.int64)
    sg = np.full((NC, ncells, T), -1.0, np.float32)
    cnt = np.zeros((NC, ncells), np.int64)
    for m in range(NC):
        s = np.nonzero(owner == m)[0]
        o = np.lexsort((segid[s], cell[s]))
        s = s[o]
        cs = cell[s]
        bounds = np.searchsorted(cs, np.arange(ncells + 1))
        for k in range(ncells):
            lo, hi = int(bounds[k]), int(bounds[k + 1])
            n = hi - lo
            if n > T:
                return None
            ga[m, k, :n] = gidx[s[lo:hi]]
            sg[m, k, :n] = segid[s[lo:hi]]
            cnt[m, k] = n
    return ga, sg, cnt


def _mm_schedule_A(sgA, cntA_max):
    """Per window w: ordered [(cell, block, sub, start, stop)], flags per
    (w, sub) across the window's whole list."""
    mmA = []
    for w in range(AW):
        nsub = 4 if w < AW - 1 else 1
        entries = []
        for c in range(NCH):
            k = w * 4 + c
            nb = -(-int(cntA_max[k]) // 128)
            for b in range(nb):
                vals = sgA[:, k, b * 128:(b + 1) * 128].ravel()
                vals = vals[vals >= 0].astype(np.int64)
                if len(vals) == 0:
                    continue
                for sub in np.unique(vals // 128):
                    s = int(sub) - w * 4
                    if s < 0 or s >= nsub:
                        return None
                    entries.append([k, b, s])
        if set(e[2] for e in entries) != set(range(nsub)):
            return None
        # PSUM zero regions are 2KB (the whole [128,512] window bank):
        # start/stop must bracket the WINDOW's mm list, not per-subwindow.
        mmA.append([(k, b, s, i == 0, i == len(entries) - 1)
                    for i, (k, b, s) in enumerate(entries)])
    return mmA


def _mm_schedule_C(sgC, cntC_max):
    """Per cell k=r*25+w: ordered [(block, sub, start, stop)], flags per
    (k, sub) within the cell."""
    mmC = []
    for r in range(2):
        for w in range(CW):
            nsub = 4 if w < CW - 1 else 2
            k = r * CW + w
            entries = []
            nb = -(-int(cntC_max[k]) // 128)
            for b in range(nb):
                vals = sgC[:, k, b * 128:(b + 1) * 128].ravel()
                vals = vals[vals >= 0].astype(np.int64)
                if len(vals) == 0:
                    continue
                for sub in np.unique(vals // 128 - w * 4):
                    s = int(sub)
                    if s < 0 or s >= nsub:
                        return None
                    entries.append([b, s])
            if set(e[1] for e in entries) != set(range(nsub)):
                return None
            mmC.append([(b, s, i == 0, i == len(entries) - 1)
                        for i, (b, s) in enumerate(entries)])
    return mmC


def _route(vertex, edges):
    eo = edges // EDGE_SH
    le = edges % EDGE_SH
    vo = vertex // NODE_SH
    lv = vertex % NODE_SH
    ch = vertex // CHUNK
    reg = (le >= REG_SPLIT).astype(np.int64)
    trow = np.where(reg == 0, eo * REG_ROWS[0] + le,
                    eo * REG_ROWS[1] + (le - REG_SPLIT))

    rA = _build_stream(eo, (le // 512) * 4 + ch, vertex - ch * CHUNK,
                       le.astype(np.float32), AW * NCH)
    if rA is None:
        return None
    gaA, sgA, cntA = rA
    rC = _build_stream(vo, reg * CW + lv // 512, trow,
                       lv.astype(np.float32), 2 * CW)
    if rC is None:
        return None
    gaC, sgC, cntC = rC

    cntA_max = cntA.max(axis=0)
    cntC_max = cntC.max(axis=0)
    # Slots beyond the static (max-over-cores) count must be NEGATIVE: the
    # SWDGE ucode requires num_idxs_reg == #(idx >= 0) and stops at the
    # last non-negative slot.  Slots in [cnt_m, cnt_max) keep idx 0 (valid
    # row; their segid is -1 so the one-hot zeroes them out).
    for k in range(AW * NCH):
        gaA[:, k, int(cntA_max[k]):] = -1
    for k in range(2 * CW):
        gaC[:, k, int(cntC_max[k]):] = -1
    mmA = _mm_schedule_A(sgA, cntA_max)
    mmC = _mm_schedule_C(sgC, cntC_max)
    if mmA is None or mmC is None:
        return None
    sig = repr((tuple(cntA_max), tuple(cntC_max), mmA, mmC))
    return dict(gaA=gaA, sgA=sgA, cntA=cntA_max, mmA=mmA,
                gaC=gaC, sgC=sgC, cntC=cntC_max, mmC=mmC, sig=sig)


def _numpy_fallback(X, vertex, edges, w_b, w_a, w_c, b_b, b_a, b_c):
    Xe = np.zeros((E, F), np.float32)
    np.add.at(Xe, edges, X[vertex])
    Xv2 = np.zeros((N, F), np.float32)
    np.add.at(Xv2, vertex, Xe[edges])
    deg = np.bincount(vertex, minlength=N).astype(np.float32)[:, None]
    Xv = np.concatenate([deg * X, Xv2], axis=1)
    center = Xv @ w_b + b_b
    aXv = np.abs(Xv)
    return (center.astype(np.float32),
            (center - (aXv @ w_a + b_a)).astype(np.float32),
            (center + (aXv @ w_c + b_c)).astype(np.float32))


# ------------------------------------------------------------- bass program
def _build_program(meta):
    from concourse import bacc, tile
    import concourse.mybir as mybir

    f32 = mybir.dt.float32
    bf16 = mybir.dt.bfloat16
    i16 = mybir.dt.int16
    Alu = mybir.AluOpType
    Abs = mybir.ActivationFunctionType.Abs
    Copy = mybir.ActivationFunctionType.Copy

    cntA, mmA = meta["cntA"], meta["mmA"]
    cntC, mmC = meta["cntC"], meta["mmC"]

    nc = bacc.Bacc(None, target_bir_lowering=False, debug=False,
                   num_devices=NC, num_swdge_queues=1)

    xq = [nc.dram_tensor(f"xq{c}", [CHUNK, F], bf16, kind="ExternalInput")
          for c in range(NCH)]
    gaA_d = nc.dram_tensor("gaA", [128, AW * NCH * 64], i16, kind="ExternalInput")
    sgA_d = nc.dram_tensor("sgA", [128, AW * NCH * 8], f32, kind="ExternalInput")
    gaC_d = nc.dram_tensor("gaC", [128, 2 * CW * 64], i16, kind="ExternalInput")
    sgC_d = nc.dram_tensor("sgC", [128, 2 * CW * 8], f32, kind="ExternalInput")
    xdt_d = nc.dram_tensor("xdt", [128, NODE_SH_P], bf16, kind="ExternalInput")
    wb1_d = nc.dram_tensor("wb1", [F, F], bf16, kind="ExternalInput")
    wb2_d = nc.dram_tensor("wb2", [F, F], bf16, kind="ExternalInput")
    wac1_d = nc.dram_tensor("wac1", [F, 2 * F], bf16, kind="ExternalInput")
    wac2_d = nc.dram_tensor("wac2", [F, 2 * F], bf16, kind="ExternalInput")
    bb_d = nc.dram_tensor("bb", [1, F], bf16, kind="ExternalInput")
    bac_d = nc.dram_tensor("bac", [1, 2 * F], bf16, kind="ExternalInput")
    iota_d = nc.dram_tensor("iota", [128, 128], f32, kind="ExternalInput")
    ident_d = nc.dram_tensor("ident", [128, 128], bf16, kind="ExternalInput")
    out3 = nc.dram_tensor("out3", [NODE_SH_P, 3 * F], bf16, kind="ExternalOutput")

    xe = [nc.dram_tensor(f"xe{r}", [REG_ROWS[r], F], bf16) for r in range(2)]
    xt = [nc.dram_tensor(f"xt{r}", [TBL_ROWS[r], F], bf16) for r in range(2)]

    with tile.TileContext(nc) as tc:
        with (
            tc.tile_pool(name="cp", bufs=1) as cp,
            tc.tile_pool(name="dp", bufs=8) as dp,
            tc.tile_pool(name="ohp", bufs=6) as ohp,
            tc.tile_pool(name="sp", bufs=4) as sp,
            tc.tile_pool(name="psw", bufs=2, space="PSUM") as psw,
            tc.tile_pool(name="psd", bufs=2, space="PSUM") as psd,
        ):
            # ---- constants / preloads
            iota = cp.tile([128, 128], f32, tag="iota")
            nc.sync.dma_start(iota[:], iota_d[:])
            identb = cp.tile([128, 128], bf16, tag="identb")
            nc.sync.dma_start(identb[:], ident_d[:])
            ones1 = cp.tile([1, F], bf16, tag="ones1")
            nc.vector.memset(ones1[:], 1.0)
            wb1s = cp.tile([F, F], bf16, tag="wb1s")
            nc.sync.dma_start(wb1s[:], wb1_d[:])
            wb2s = cp.tile([F, F], bf16, tag="wb2s")
            nc.sync.dma_start(wb2s[:], wb2_d[:])
            wac1s = cp.tile([F, 2 * F], bf16, tag="wac1s")
            nc.sync.dma_start(wac1s[:], wac1_d[:])
            wac2s = cp.tile([F, 2 * F], bf16, tag="wac2s")
            nc.sync.dma_start(wac2s[:], wac2_d[:])
            bbs = cp.tile([1, F], bf16, tag="bbs")
            nc.sync.dma_start(bbs[:], bb_d[:])
            bacs = cp.tile([1, 2 * F], bf16, tag="bacs")
            nc.sync.dma_start(bacs[:], bac_d[:])
            gaA_sb = cp.tile([128, AW * NCH * 64], i16, tag="gaA_sb")
            nc.scalar.dma_start(gaA_sb[:], gaA_d[:])
            sgA_sb = cp.tile([128, AW * NCH * 8], f32, tag="sgA_sb")
            nc.scalar.dma_start(sgA_sb[:], sgA_d[:])
            gaC_sb = cp.tile([128, 2 * CW * 64], i16, tag="gaC_sb")
            nc.scalar.dma_start(gaC_sb[:], gaC_d[:])
            sgC_sb = cp.tile([128, 2 * CW * 8], f32, tag="sgC_sb")
            nc.scalar.dma_start(sgC_sb[:], sgC_d[:])
            xdt_sb = cp.tile([128, NODE_SH_P], bf16, tag="xdt_sb")
            nc.sync.dma_start(xdt_sb[:], xdt_d[:])
            xv2sb = cp.tile([128, NODE_SH_P], bf16, tag="xv2sb")

            # bias_b broadcast tile (ones-matmul trick).  All psd tiles are
            # padded to a full 2KB PSUM bank: start_tensor_calc marks the
            # whole bank pending-zero, so co-resident tiles would corrupt
            # each other.
            psb = psd.tile([128, 512], f32, tag="pscen")
            nc.tensor.matmul(psb[:, :F], ones1[:], bbs[:], start=True, stop=True)
            bcb = cp.tile([128, F], f32, tag="bcb")
            nc.vector.tensor_copy(bcb[:], psb[:, :F])

            zeros = cp.tile([128, F], bf16, tag="zeros")
            nc.vector.memset(zeros[:], 0.0)

            def tail_memset(dat, cnt):
                # Slots >= the gather's runtime count are never written by
                # the DMA (trailing -1 idxs); zero the boundary block's tail
                # so the one-hot matmuls read finite data.  Engine ops can
                # only start at partition 0/32/64/96, so use a DMA copy.
                pb, pp = cnt // 128, cnt % 128
                if pp:
                    nc.sync.dma_start(dat[pp:128, pb, :], zeros[pp:128, :])

            # ---- stage A: Xe^T windows
            for w in range(AW):
                nsub = 4 if w < AW - 1 else 1
                wsz = nsub * 128
                ps = psw.tile([128, 512], f32, tag="psA")
                dats = {}
                for c in range(NCH):
                    k = w * 4 + c
                    if cntA[k] == 0:
                        continue
                    dat = dp.tile([128, 8, F], bf16, tag="dA")
                    nc.gpsimd.dma_gather(
                        dat[:], xq[c].ap(), gaA_sb[:, k * 64:(k + 1) * 64],
                        T, int(cntA[k]), F)
                    tail_memset(dat, int(cntA[k]))
                    dats[c] = dat
                for (k, b, s, st, sp_) in mmA[w]:
                    oh = ohp.tile([128, 128], bf16, tag="oh")
                    nc.vector.tensor_scalar(
                        oh[:], iota[:], sgA_sb[:, k * 8 + b:k * 8 + b + 1],
                        float(-(w * 512 + s * 128)),
                        op0=Alu.subtract, op1=Alu.is_equal)
                    nc.tensor.matmul(ps[:, s * 128:(s + 1) * 128],
                                     dats[k % 4][:, b, :], oh[:],
                                     start=st, stop=sp_)
                xs = sp.tile([128, 512], bf16, tag="xs")
                nc.scalar.activation(xs[:, :wsz], ps[:, :wsz], Copy)
                for s in range(nsub):
                    S = w * 4 + s
                    pt = psd.tile([128, 1024], bf16, tag="pscen")
                    nc.tensor.transpose(pt[:, :128], xs[:, s * 128:(s + 1) * 128],
                                        identb[:])
                    xo = sp.tile([128, 128], bf16, tag="xo")
                    nc.vector.tensor_copy(xo[:], pt[:, :128])
                    r, row = (0, S * 128) if S < 24 else (1, S * 128 - 3072)
                    nc.sync.dma_start(xe[r][row:row + 128, :], xo[:])
                if w == 5:
                    nc.gpsimd.collective_compute(
                        "AllGather", Alu.bypass,
                        replica_groups=[list(range(NC))],
                        ins=[xe[0].ap().opt()], outs=[xt[0].ap().opt()])
                if w == AW - 1:
                    nc.gpsimd.collective_compute(
                        "AllGather", Alu.bypass,
                        replica_groups=[list(range(NC))],
                        ins=[xe[1].ap().opt()], outs=[xt[1].ap().opt()])

            # ---- stage C: Xv2^T windows + fused dense head
            for r in range(2):
                for w in range(CW):
                    nsub = 4 if w < CW - 1 else 2
                    wsz = nsub * 128
                    k = r * CW + w
                    ps = psw.tile([128, 512], f32, tag="psC")
                    dat = dp.tile([128, 8, F], bf16, tag="dC")
                    nc.gpsimd.dma_gather(
                        dat[:], xt[r].ap(), gaC_sb[:, k * 64:(k + 1) * 64],
                        T, int(cntC[k]), F)
                    tail_memset(dat, int(cntC[k]))
                    for (b, s, st, sp_) in mmC[k]:
                        oh = ohp.tile([128, 128], bf16, tag="oh")
                        nc.vector.tensor_scalar(
                            oh[:], iota[:], sgC_sb[:, k * 8 + b:k * 8 + b + 1],
                            float(-(w * 512 + s * 128)),
                            op0=Alu.subtract, op1=Alu.is_equal)
                        nc.tensor.matmul(ps[:, s * 128:(s + 1) * 128],
                                         dat[:, b, :], oh[:],
                                         start=st, stop=sp_)
                    sl = xv2sb[:, w * 512:w * 512 + wsz]
                    if r == 0:
                        nc.vector.tensor_copy(sl, ps[:, :wsz])
                        continue
                    nc.vector.tensor_add(sl, sl, ps[:, :wsz])
                    for g in range(w * 4, min(w * 4 + nsub, NGRP)):
                        xd = xdt_sb[:, g * 128:(g + 1) * 128]
                        v2 = xv2sb[:, g * 128:(g + 1) * 128]
                        axd = sp.tile([128, 128], bf16, tag="axd")
                        nc.scalar.activation(axd[:], xd, Abs)
                        av2 = sp.tile([128, 128], bf16, tag="av2")
                        nc.scalar.activation(av2[:], v2, Abs)
                        pc = psd.tile([128, 512], f32, tag="pscen")
                        nc.tensor.matmul(pc[:, :F], xd, wb1s[:], start=True, stop=False)
                        nc.tensor.matmul(pc[:, :F], v2, wb2s[:], start=False, stop=True)
                        pl = psd.tile([128, 512], f32, tag="pslr")
                        nc.tensor.matmul(pl[:, :2 * F], axd[:], wac1s[:],
                                         start=True, stop=False)
                        nc.tensor.matmul(pl[:, :2 * F], av2[:], wac2s[:],
                                         start=False, stop=False)
                        nc.tensor.matmul(pl[:, :2 * F], ones1[:], bacs[:],
                                         start=False, stop=True)
                        ot = sp.tile([128, 3 * F], bf16, tag="ot")
                        nc.vector.tensor_add(ot[:, 0:F], bcb[:], pc[:, :F])
                        nc.vector.tensor_sub(ot[:, F:2 * F], ot[:, 0:F], pl[:, 0:F])
                        nc.vector.tensor_add(ot[:, 2 * F:3 * F], ot[:, 0:F],
                                             pl[:, F:2 * F])
                        rows = min(128, NODE_SH - g * 128)
                        nc.sync.dma_start(out3[g * 128:g * 128 + rows, :],
                                          ot[:rows, :])

    nc.compile()
    return nc


# ------------------------------------------------------------------- driver
def kernel(X, vertex, edges, X0, n_edges, w_b, w_a, w_c, b_b, b_a, b_c):
    from concourse.bass_utils import run_bass_kernel_spmd
    import ml_dtypes

    bf = ml_dtypes.bfloat16
    X = np.ascontiguousarray(np.asarray(X, dtype=np.float32))
    vertex = np.asarray(vertex).astype(np.int64)
    edges = np.asarray(edges).astype(np.int64)
    w_b = np.asarray(w_b, dtype=np.float32)
    w_a = np.asarray(w_a, dtype=np.float32)
    w_c = np.asarray(w_c, dtype=np.float32)
    b_b = np.asarray(b_b, dtype=np.float32).reshape(1, F)
    b_a = np.asarray(b_a, dtype=np.float32).reshape(1, F)
    b_c = np.asarray(b_c, dtype=np.float32).reshape(1, F)

    meta = _route(vertex, edges)
    if meta is None:
        return _numpy_fallback(X, vertex, edges, w_b, w_a, w_c, b_b, b_a, b_c)

    if _STATE.get("sig") != meta["sig"]:
        _STATE["nc"] = _build_program(meta)
        _STATE["sig"] = meta["sig"]
    nc = _STATE["nc"]

    Xbf = X.astype(bf)
    deg = np.bincount(vertex, minlength=N).astype(np.float32)
    XD = (X * deg[:, None]).astype(np.float32)

    iota_np = np.ascontiguousarray(
        np.tile(np.arange(128, dtype=np.float32), (128, 1)))
    ident_np = np.ascontiguousarray(np.eye(128, dtype=np.float32).astype(bf))
    wb1 = np.ascontiguousarray(w_b[:F].astype(bf))
    wb2 = np.ascontiguousarray(w_b[F:].astype(bf))
    wac1 = np.ascontiguousarray(
        np.concatenate([w_a[:F], w_c[:F]], axis=1).astype(bf))
    wac2 = np.ascontiguousarray(
        np.concatenate([w_a[F:], w_c[F:]], axis=1).astype(bf))
    bb = np.ascontiguousarray(b_b.astype(bf))
    bac = np.ascontiguousarray(np.concatenate([b_a, b_c], axis=1).astype(bf))

    in_maps = []
    for m in range(NC):
        xdm = np.zeros((128, NODE_SH_P), np.float32)
        xdm[:, :NODE_SH] = XD[m * NODE_SH:(m + 1) * NODE_SH].T
        im = {
            "gaA": _wrap16(meta["gaA"][m]),
            "sgA": _seg_img(meta["sgA"][m]),
            "gaC": _wrap16(meta["gaC"][m]),
            "sgC": _seg_img(meta["sgC"][m]),
            "xdt": np.ascontiguousarray(xdm.astype(bf)),
            "wb1": wb1, "wb2": wb2, "wac1": wac1, "wac2": wac2,
            "bb": bb, "bac": bac,
            "iota": iota_np, "ident": ident_np,
        }
        for c in range(NCH):
            im[f"xq{c}"] = np.ascontiguousarray(Xbf[c * CHUNK:(c + 1) * CHUNK])
        in_maps.append(im)

    res = run_bass_kernel_spmd(nc, in_maps, list(range(NC)))
    out = np.concatenate(
        [np.asarray(res.results[m]["out3"])[:NODE_SH].astype(np.float32)
         for m in range(NC)], axis=0)
    return (np.ascontiguousarray(out[:, 0:F]),
            np.ascontiguousarray(out[:, F:2 * F]),
            np.ascontiguousarray(out[:, 2 * F:3 * F]))


# revision 16
# speedup vs baseline: 1.2363x; 1.2363x over previous
"""Trainium2 Bass kernel v2 for nn_CrispToFuzzyConv (hypergraph message passing).

Math (see reference):
  Xe   = segment_sum(X[vertex], edges, E)                 # round 1
  Xv   = concat([deg * X, Xv2]),  Xv2 = segment_sum(Xe[edges], vertex, N)
  center = Xv @ w_b + b_b
  HL = center - (|Xv| @ w_a + b_a)
  HR = center + (|Xv| @ w_c + b_c)

v2 strategy (vs v1 which used dma_scatter_add):
  - NO scatter-adds.  Segment sums are computed as one-hot matmuls on the
    PE: gathered token rows [128tok x F] are contracted against on-chip
    one-hot matrices built with one DVE op each:
        onehot = ((iota - segid) == -(window_base))    # tensor_scalar
    For stage A the PSUM window is Xe^T [F x 512 edge-cols]; for stage C
    it is Xv2^T [F x 512 node-cols] which feeds the dense head directly
    as matmul lhsT (no transposes in the dense path; the host supplies
    deg-scaled X^T).
  - Everything is bf16 on the wire (gather rows are 256B packets, the
    AllGather payload halves); PSUM accumulates in f32.
  - SPMD: one program for all 8 cores.  The schedule is static, sized by
    the max token count over cores per (window, chunk/region) cell; each
    cell is one 1024-slot gather call whose runtime count (num_idxs_reg)
    is that max, so padding costs ~3% packets, not 30%.
  - Stage C accumulates region passes into an SBUF-resident Xv2 table
    (24.5KB/partition) so PSUM windows stay short-lived; the dense head
    runs fused per 128-node group right after its window closes.
  - Two AllGathers (edge regions) pipeline behind stage A / stage C
    descriptor generation.

Hardware constraints baked in (from v1 + this session's trace analysis):
  - dma_gather idx dtype is int16 -> gather tables <= 32767 rows: X is
    split in 4 chunks of 25000, Xe tables in 2 regions (24576/25600 rows)
  - elem_size_bytes % 256 == 0 -> bf16 F=128 rows (256B) are legal
  - gather output layout: token i -> partition i%128, col-block i//128,
    which is exactly the PE contraction layout
  - SWDGE data packets ride all 16 DMA engines at ~45ns/packet: the
    bottleneck is PACKET COUNT (= tokens), so padding is minimized and
    all other traffic is kept off the critical path
  - stale SBUF slots beyond a call's runtime count are made safe by
    memsetting the gather pool buffers once at startup (0.0 * onehot 0
    contributes nothing; avoids NaN poisoning)
"""

import numpy as np

# ---------------------------------------------------------------- constants
N = 100000
E = 50000
NNZ = 300000
F = 128
NC = 8

NODE_SH = 12500              # nodes per core
NODE_SH_P = 12544            # 98 groups of 128
NGRP = 98
CW = 25                      # 512-node windows (last = 256)

EDGE_SH = 6250               # edges per core
EDGE_SH_P = 6272             # 49 subwindows of 128
AW = 13                      # 512-edge windows (last = 128)
NCH = 4
CHUNK = 25000                # X chunk rows (int16 gather limit)

REG_SPLIT = 3072             # local edge rows [0,3072) -> region 0
REG_ROWS = (3072, 3200)      # padded local rows per region
TBL_ROWS = (NC * 3072, NC * 3200)

T = 1024                     # slots per gather call (8 blocks of 128)

_STATE = {}


# ---------------------------------------------------------------- host side
def _wrap16(idx):
    """[ncall, 1024] int -> [128, ncall*64] int16 SBUF image (idx i at
    partition i%16, col i//16; replicated across the 8 groups of 16)."""
    ncall = idx.shape[0]
    t = idx.reshape(ncall, 64, 16).transpose(0, 2, 1).astype(np.int16)
    t = np.tile(t, (1, 8, 1))                      # [ncall, 128, 64]
    return np.ascontiguousarray(t.transpose(1, 0, 2).reshape(128, ncall * 64))


def _seg_img(seg):
    """[ncall, 1024] f32 -> [128, ncall*8] f32 SBUF image (slot b*128+p of
    call k -> partition p, col k*8+b)."""
    ncall = seg.shape[0]
    t = seg.reshape(ncall, 8, 128).transpose(2, 0, 1).astype(np.float32)
    return np.ascontiguousarray(t.reshape(128, ncall * 8))


def _build_stream(owner, cell, gidx, segid, ncells):
    """Pack tokens into per-(core, cell) sorted slot arrays."""
    ga = np.zeros((NC, ncells, T), np.int64)
    sg = np.full((NC, ncells, T), -1.0, np.float32)
    cnt = np.zeros((NC, ncells), np.int64)
    for m in range(NC):
        s = np.nonzero(owner == m)[0]
        o = np.lexsort((segid[s], cell[s]))
        s = s[o]
        cs = cell[s]
        bounds = np.searchsorted(cs, np.arange(ncells + 1))
        for k in range(ncells):
            lo, hi = int(bounds[k]), int(bounds[k + 1])
            n = hi - lo
            if n > T:
                return None
            ga[m, k, :n] = gidx[s[lo:hi]]
            sg[m, k, :n] = segid[s[lo:hi]]
            cnt[m, k] = n
    return ga, sg, cnt


def _mm_schedule_A(sgA, cntA_max):
    """Per window w: ordered [(cell, block, sub, start, stop)], flags per
    (w, sub) across the window's whole list."""
    mmA = []
    for w in range(AW):
        nsub = 4 if w < AW - 1 else 1
        entries = []
        for c in range(NCH):
            k = w * 4 + c
            nb = -(-int(cntA_max[k]) // 128)
            for b in range(nb):
                vals = sgA[:, k, b * 128:(b + 1) * 128].ravel()
                vals = vals[vals >= 0].astype(np.int64)
                if len(vals) == 0:
                    continue
                for sub in np.unique(vals // 128):
                    s = int(sub) - w * 4
                    if s < 0 or s >= nsub:
                        return None
                    entries.append([k, b, s])
        if set(e[2] for e in entries) != set(range(nsub)):
            return None
        # PSUM zero regions are 2KB (the whole [128,512] window bank):
        # start/stop must bracket the WINDOW's mm list, not per-subwindow.
        mmA.append([(k, b, s, i == 0, i == len(entries) - 1)
                    for i, (k, b, s) in enumerate(entries)])
    return mmA


def _mm_schedule_C(sgC, cntC_max):
    """Per cell k=r*25+w: ordered [(block, sub, start, stop)], flags per
    (k, sub) within the cell."""
    mmC = []
    for r in range(2):
        for w in range(CW):
            nsub = 4 if w < CW - 1 else 2
            k = r * CW + w
            entries = []
            nb = -(-int(cntC_max[k]) // 128)
            for b in range(nb):
                vals = sgC[:, k, b * 128:(b + 1) * 128].ravel()
                vals = vals[vals >= 0].astype(np.int64)
                if len(vals) == 0:
                    continue
                for sub in np.unique(vals // 128 - w * 4):
                    s = int(sub)
                    if s < 0 or s >= nsub:
                        return None
                    entries.append([b, s])
            if set(e[1] for e in entries) != set(range(nsub)):
                return None
            mmC.append([(b, s, i == 0, i == len(entries) - 1)
                        for i, (b, s) in enumerate(entries)])
    return mmC


def _route(vertex, edges):
    eo = edges // EDGE_SH
    le = edges % EDGE_SH
    vo = vertex // NODE_SH
    lv = vertex % NODE_SH
    ch = vertex // CHUNK
    reg = (le >= REG_SPLIT).astype(np.int64)
    trow = np.where(reg == 0, eo * REG_ROWS[0] + le,
                    eo * REG_ROWS[1] + (le - REG_SPLIT))

    rA = _build_stream(eo, (le // 512) * 4 + ch, vertex - ch * CHUNK,
                       le.astype(np.float32), AW * NCH)
    if rA is None:
        return None
    gaA, sgA, cntA = rA
    rC = _build_stream(vo, reg * CW + lv // 512, trow,
                       lv.astype(np.float32), 2 * CW)
    if rC is None:
        return None
    gaC, sgC, cntC = rC

    cntA_max = cntA.max(axis=0)
    cntC_max = cntC.max(axis=0)
    # Slots beyond the static (max-over-cores) count must be NEGATIVE: the
    # SWDGE ucode requires num_idxs_reg == #(idx >= 0) and stops at the
    # last non-negative slot.  Slots in [cnt_m, cnt_max) keep idx 0 (valid
    # row; their segid is -1 so the one-hot zeroes them out).
    for k in range(AW * NCH):
        gaA[:, k, int(cntA_max[k]):] = -1
    for k in range(2 * CW):
        gaC[:, k, int(cntC_max[k]):] = -1
    mmA = _mm_schedule_A(sgA, cntA_max)
    mmC = _mm_schedule_C(sgC, cntC_max)
    if mmA is None or mmC is None:
        return None
    sig = repr((tuple(cntA_max), tuple(cntC_max), mmA, mmC))
    return dict(gaA=gaA, sgA=sgA, cntA=cntA_max, mmA=mmA,
                gaC=gaC, sgC=sgC, cntC=cntC_max, mmC=mmC, sig=sig)


def _numpy_fallback(X, vertex, edges, w_b, w_a, w_c, b_b, b_a, b_c):
    Xe = np.zeros((E, F), np.float32)
    np.add.at(Xe, edges, X[vertex])
    Xv2 = np.zeros((N, F), np.float32)
    np.add.at(Xv2, vertex, Xe[edges])
    deg = np.bincount(vertex, minlength=N).astype(np.float32)[:, None]
    Xv = np.concatenate([deg * X, Xv2], axis=1)
    center = Xv @ w_b + b_b
    aXv = np.abs(Xv)
    return (center.astype(np.float32),
            (center - (aXv @ w_a + b_a)).astype(np.float32),
            (center + (aXv @ w_c + b_c)).astype(np.float32))


# ------------------------------------------------------------- bass program
def _build_program(meta):
    from concourse import bacc, tile
    import concourse.mybir as mybir

    f32 = mybir.dt.float32
    bf16 = mybir.dt.bfloat16
    i16 = mybir.dt.int16
    Alu = mybir.AluOpType
    Abs = mybir.ActivationFunctionType.Abs
    Copy = mybir.ActivationFunctionType.Copy

    cntA, mmA = meta["cntA"], meta["mmA"]
    cntC, mmC = meta["cntC"], meta["mmC"]

    NQ = 4
    nc = bacc.Bacc(None, target_bir_lowering=False, debug=False,
                   num_devices=NC, num_swdge_queues=NQ)

    xq = [nc.dram_tensor(f"xq{c}", [CHUNK, F], bf16, kind="ExternalInput")
          for c in range(NCH)]
    gaA_d = nc.dram_tensor("gaA", [128, AW * NCH * 64], i16, kind="ExternalInput")
    sgA_d = nc.dram_tensor("sgA", [128, AW * NCH * 8], f32, kind="ExternalInput")
    gaC_d = nc.dram_tensor("gaC", [128, 2 * CW * 64], i16, kind="ExternalInput")
    sgC_d = nc.dram_tensor("sgC", [128, 2 * CW * 8], f32, kind="ExternalInput")
    xdt_d = nc.dram_tensor("xdt", [128, NODE_SH_P], bf16, kind="ExternalInput")
    wb1_d = nc.dram_tensor("wb1", [F, F], bf16, kind="ExternalInput")
    wb2_d = nc.dram_tensor("wb2", [F, F], bf16, kind="ExternalInput")
    wac1_d = nc.dram_tensor("wac1", [F, 2 * F], bf16, kind="ExternalInput")
    wac2_d = nc.dram_tensor("wac2", [F, 2 * F], bf16, kind="ExternalInput")
    bb_d = nc.dram_tensor("bb", [1, F], bf16, kind="ExternalInput")
    bac_d = nc.dram_tensor("bac", [1, 2 * F], bf16, kind="ExternalInput")
    iota_d = nc.dram_tensor("iota", [128, 128], f32, kind="ExternalInput")
    ident_d = nc.dram_tensor("ident", [128, 128], bf16, kind="ExternalInput")
    out3 = nc.dram_tensor("out3", [NODE_SH_P, 3 * F], bf16, kind="ExternalOutput")

    xe = [nc.dram_tensor(f"xe{r}", [REG_ROWS[r], F], bf16) for r in range(2)]
    xt = [nc.dram_tensor(f"xt{r}", [TBL_ROWS[r], F], bf16) for r in range(2)]

    with tile.TileContext(nc) as tc:
        with (
            tc.tile_pool(name="cp", bufs=1) as cp,
            tc.tile_pool(name="dp", bufs=8) as dp,
            tc.tile_pool(name="ohp", bufs=6) as ohp,
            tc.tile_pool(name="sp", bufs=4) as sp,
            tc.tile_pool(name="psw", bufs=2, space="PSUM") as psw,
            tc.tile_pool(name="psd", bufs=2, space="PSUM") as psd,
        ):
            # ---- constants / preloads
            iota = cp.tile([128, 128], f32, tag="iota")
            nc.sync.dma_start(iota[:], iota_d[:])
            identb = cp.tile([128, 128], bf16, tag="identb")
            nc.sync.dma_start(identb[:], ident_d[:])
            ones1 = cp.tile([1, F], bf16, tag="ones1")
            nc.vector.memset(ones1[:], 1.0)
            wb1s = cp.tile([F, F], bf16, tag="wb1s")
            nc.sync.dma_start(wb1s[:], wb1_d[:])
            wb2s = cp.tile([F, F], bf16, tag="wb2s")
            nc.sync.dma_start(wb2s[:], wb2_d[:])
            wac1s = cp.tile([F, 2 * F], bf16, tag="wac1s")
            nc.sync.dma_start(wac1s[:], wac1_d[:])
            wac2s = cp.tile([F, 2 * F], bf16, tag="wac2s")
            nc.sync.dma_start(wac2s[:], wac2_d[:])
            bbs = cp.tile([1, F], bf16, tag="bbs")
            nc.sync.dma_start(bbs[:], bb_d[:])
            bacs = cp.tile([1, 2 * F], bf16, tag="bacs")
            nc.sync.dma_start(bacs[:], bac_d[:])
            gaA_sb = cp.tile([128, AW * NCH * 64], i16, tag="gaA_sb")
            nc.scalar.dma_start(gaA_sb[:], gaA_d[:])
            sgA_sb = cp.tile([128, AW * NCH * 8], f32, tag="sgA_sb")
            nc.scalar.dma_start(sgA_sb[:], sgA_d[:])
            gaC_sb = cp.tile([128, 2 * CW * 64], i16, tag="gaC_sb")
            nc.scalar.dma_start(gaC_sb[:], gaC_d[:])
            sgC_sb = cp.tile([128, 2 * CW * 8], f32, tag="sgC_sb")
            nc.scalar.dma_start(sgC_sb[:], sgC_d[:])
            xdt_sb = cp.tile([128, NODE_SH_P], bf16, tag="xdt_sb")
            nc.sync.dma_start(xdt_sb[:], xdt_d[:])
            xv2sb = cp.tile([128, NODE_SH_P], bf16, tag="xv2sb")

            # bias_b broadcast tile (ones-matmul trick).  All psd tiles are
            # padded to a full 2KB PSUM bank: start_tensor_calc marks the
            # whole bank pending-zero, so co-resident tiles would corrupt
            # each other.
            psb = psd.tile([128, 512], f32, tag="pscen")
            nc.tensor.matmul(psb[:, :F], ones1[:], bbs[:], start=True, stop=True)
            bcb = cp.tile([128, F], f32, tag="bcb")
            nc.vector.tensor_copy(bcb[:], psb[:, :F])

            zeros = cp.tile([128, F], bf16, tag="zeros")
            nc.vector.memset(zeros[:], 0.0)

            def tail_memset(dat, cnt):
                # Slots >= the gather's runtime count are never written by
                # the DMA (trailing -1 idxs); zero the boundary block's tail
                # so the one-hot matmuls read finite data.  Engine ops can
                # only start at partition 0/32/64/96, so use a DMA copy.
                pb, pp = cnt // 128, cnt % 128
                if pp:
                    nc.sync.dma_start(dat[pp:128, pb, :], zeros[pp:128, :])

            # ---- stage A: Xe^T windows
            for w in range(AW):
                nsub = 4 if w < AW - 1 else 1
                wsz = nsub * 128
                ps = psw.tile([128, 512], f32, tag="psA")
                dats = {}
                for c in range(NCH):
                    k = w * 4 + c
                    if cntA[k] == 0:
                        continue
                    dat = dp.tile([128, 8, F], bf16, tag="dA")
                    nc.gpsimd.dma_gather(
                        dat[:], xq[c].ap(), gaA_sb[:, k * 64:(k + 1) * 64],
                        T, int(cntA[k]), F)
                    tail_memset(dat, int(cntA[k]))
                    dats[c] = dat
                for (k, b, s, st, sp_) in mmA[w]:
                    oh = ohp.tile([128, 128], bf16, tag="oh")
                    nc.vector.tensor_scalar(
                        oh[:], iota[:], sgA_sb[:, k * 8 + b:k * 8 + b + 1],
                        float(-(w * 512 + s * 128)),
                        op0=Alu.subtract, op1=Alu.is_equal)
                    nc.tensor.matmul(ps[:, s * 128:(s + 1) * 128],
                                     dats[k % 4][:, b, :], oh[:],
                                     start=st, stop=sp_)
                xs = sp.tile([128, 512], bf16, tag="xs")
                nc.scalar.activation(xs[:, :wsz], ps[:, :wsz], Copy)
                for s in range(nsub):
                    S = w * 4 + s
                    pt = psd.tile([128, 1024], bf16, tag="pscen")
                    nc.tensor.transpose(pt[:, :128], xs[:, s * 128:(s + 1) * 128],
                                        identb[:])
                    xo = sp.tile([128, 128], bf16, tag="xo")
                    nc.vector.tensor_copy(xo[:], pt[:, :128])
                    r, row = (0, S * 128) if S < 24 else (1, S * 128 - 3072)
                    nc.sync.dma_start(xe[r][row:row + 128, :], xo[:])
                if w == 5:
                    nc.gpsimd.collective_compute(
                        "AllGather", Alu.bypass,
                        replica_groups=[list(range(NC))],
                        ins=[xe[0].ap().opt()], outs=[xt[0].ap().opt()])
                if w == AW - 1:
                    nc.gpsimd.collective_compute(
                        "AllGather", Alu.bypass,
                        replica_groups=[list(range(NC))],
                        ins=[xe[1].ap().opt()], outs=[xt[1].ap().opt()])

            # ---- stage C: Xv2^T windows + fused dense head
            for r in range(2):
                for w in range(CW):
                    nsub = 4 if w < CW - 1 else 2
                    wsz = nsub * 128
                    k = r * CW + w
                    ps = psw.tile([128, 512], f32, tag="psC")
                    dat = dp.tile([128, 8, F], bf16, tag="dC")
                    nc.gpsimd.dma_gather(
                        dat[:], xt[r].ap(), gaC_sb[:, k * 64:(k + 1) * 64],
                        T, int(cntC[k]), F)
                    tail_memset(dat, int(cntC[k]))
                    for (b, s, st, sp_) in mmC[k]:
                        oh = ohp.tile([128, 128], bf16, tag="oh")
                        nc.vector.tensor_scalar(
                            oh[:], iota[:], sgC_sb[:, k * 8 + b:k * 8 + b + 1],
                            float(-(w * 512 + s * 128)),
                            op0=Alu.subtract, op1=Alu.is_equal)
                        nc.tensor.matmul(ps[:, s * 128:(s + 1) * 128],
                                         dat[:, b, :], oh[:],
                                         start=st, stop=sp_)
                    sl = xv2sb[:, w * 512:w * 512 + wsz]
                    if r == 0:
                        nc.vector.tensor_copy(sl, ps[:, :wsz])
                        continue
                    nc.vector.tensor_add(sl, sl, ps[:, :wsz])
                    for g in range(w * 4, min(w * 4 + nsub, NGRP)):
                        xd = xdt_sb[:, g * 128:(g + 1) * 128]
                        v2 = xv2sb[:, g * 128:(g + 1) * 128]
                        axd = sp.tile([128, 128], bf16, tag="axd")
                        nc.scalar.activation(axd[:], xd, Abs)
                        av2 = sp.tile([128, 128], bf16, tag="av2")
                        nc.scalar.activation(av2[:], v2, Abs)
                        pc = psd.tile([128, 512], f32, tag="pscen")
                        nc.tensor.matmul(pc[:, :F], xd, wb1s[:], start=True, stop=False)
                        nc.tensor.matmul(pc[:, :F], v2, wb2s[:], start=False, stop=True)
                        pl = psd.tile([128, 512], f32, tag="pslr")
                        nc.tensor.matmul(pl[:, :2 * F], axd[:], wac1s[:],
                                         start=True, stop=False)
                        nc.tensor.matmul(pl[:, :2 * F], av2[:], wac2s[:],
                                         start=False, stop=False)
                        nc.tensor.matmul(pl[:, :2 * F], ones1[:], bacs[:],
                                         start=False, stop=True)
                        ot = sp.tile([128, 3 * F], bf16, tag="ot")
                        nc.vector.tensor_add(ot[:, 0:F], bcb[:], pc[:, :F])
                        nc.vector.tensor_sub(ot[:, F:2 * F], ot[:, 0:F], pl[:, 0:F])
                        nc.vector.tensor_add(ot[:, 2 * F:3 * F], ot[:, 0:F],
                                             pl[:, F:2 * F])
                        rows = min(128, NODE_SH - g * 128)
                        nc.sync.dma_start(out3[g * 128:g * 128 + rows, :],
                                          ot[:rows, :])

    # SWDGE queue assignment must match the DMASW semaphore lane the Tile
    # sem-assignment pass gave each gather (lanes rotate over Pool-engine
    # DMA insts in SCHEDULED order, which differs from emission order; a
    # lane's semaphore is locked to one queue).  queue = lane % NQ keeps
    # every lane on exactly one queue while spreading descriptor-gen work
    # across all 4 Q7 ucode workers.
    from concourse.tile_sem_assignment import PROC_NAME_TO_IDX
    idx2lane = {PROC_NAME_TO_IDX[f"DMASW{i}"]: i for i in range(8)}
    for insts in tc.ordered_instructions_by_block.values():
        for inst in insts:
            if isinstance(inst, mybir.InstDMAGatherAnt):
                lane = idx2lane.get(getattr(inst, "bass_scheduled_proc", -1))
                if lane is not None:
                    inst.queue_num = lane % NQ

    nc.compile()
    return nc


# ------------------------------------------------------------------- driver
def kernel(X, vertex, edges, X0, n_edges, w_b, w_a, w_c, b_b, b_a, b_c):
    from concourse.bass_utils import run_bass_kernel_spmd
    import ml_dtypes

    bf = ml_dtypes.bfloat16
    X = np.ascontiguousarray(np.asarray(X, dtype=np.float32))
    vertex = np.asarray(vertex).astype(np.int64)
    edges = np.asarray(edges).astype(np.int64)
    w_b = np.asarray(w_b, dtype=np.float32)
    w_a = np.asarray(w_a, dtype=np.float32)
    w_c = np.asarray(w_c, dtype=np.float32)
    b_b = np.asarray(b_b, dtype=np.float32).reshape(1, F)
    b_a = np.asarray(b_a, dtype=np.float32).reshape(1, F)
    b_c = np.asarray(b_c, dtype=np.float32).reshape(1, F)

    meta = _route(vertex, edges)
    if meta is None:
        return _numpy_fallback(X, vertex, edges, w_b, w_a, w_c, b_b, b_a, b_c)

    if _STATE.get("sig") != meta["sig"]:
        _STATE["nc"] = _build_program(meta)
        _STATE["sig"] = meta["sig"]
    nc = _STATE["nc"]

    Xbf = X.astype(bf)
    deg = np.bincount(vertex, minlength=N).astype(np.float32)
    XD = (X * deg[:, None]).astype(np.float32)

    iota_np = np.ascontiguousarray(
        np.tile(np.arange(128, dtype=np.float32), (128, 1)))
    ident_np = np.ascontiguousarray(np.eye(128, dtype=np.float32).astype(bf))
    wb1 = np.ascontiguousarray(w_b[:F].astype(bf))
    wb2 = np.ascontiguousarray(w_b[F:].astype(bf))
    wac1 = np.ascontiguousarray(
        np.concatenate([w_a[:F], w_c[:F]], axis=1).astype(bf))
    wac2 = np.ascontiguousarray(
        np.concatenate([w_a[F:], w_c[F:]], axis=1).astype(bf))
    bb = np.ascontiguousarray(b_b.astype(bf))
    bac = np.ascontiguousarray(np.concatenate([b_a, b_c], axis=1).astype(bf))

    in_maps = []
    for m in range(NC):
        xdm = np.zeros((128, NODE_SH_P), np.float32)
        xdm[:, :NODE_SH] = XD[m * NODE_SH:(m + 1) * NODE_SH].T
        im = {
            "gaA": _wrap16(meta["gaA"][m]),
            "sgA": _seg_img(meta["sgA"][m]),
            "gaC": _wrap16(meta["gaC"][m]),
            "sgC": _seg_img(meta["sgC"][m]),
            "xdt": np.ascontiguousarray(xdm.astype(bf)),
            "wb1": wb1, "wb2": wb2, "wac1": wac1, "wac2": wac2,
            "bb": bb, "bac": bac,
            "iota": iota_np, "ident": ident_np,
        }
        for c in range(NCH):
            im[f"xq{c}"] = np.ascontiguousarray(Xbf[c * CHUNK:(c + 1) * CHUNK])
        in_maps.append(im)

    res = run_bass_kernel_spmd(nc, in_maps, list(range(NC)))
    out = np.concatenate(
        [np.asarray(res.results[m]["out3"])[:NODE_SH].astype(np.float32)
         for m in range(NC)], axis=0)
    return (np.ascontiguousarray(out[:, 0:F]),
            np.ascontiguousarray(out[:, F:2 * F]),
            np.ascontiguousarray(out[:, 2 * F:3 * F]))


# revision 18
# speedup vs baseline: 2.1817x; 1.7647x over previous
"""Trainium2 Bass kernel v2 for nn_CrispToFuzzyConv (hypergraph message passing).

Math (see reference):
  Xe   = segment_sum(X[vertex], edges, E)                 # round 1
  Xv   = concat([deg * X, Xv2]),  Xv2 = segment_sum(Xe[edges], vertex, N)
  center = Xv @ w_b + b_b
  HL = center - (|Xv| @ w_a + b_a)
  HR = center + (|Xv| @ w_c + b_c)

v2 strategy (vs v1 which used dma_scatter_add):
  - NO scatter-adds.  Segment sums are computed as one-hot matmuls on the
    PE: gathered token rows [128tok x F] are contracted against on-chip
    one-hot matrices.  All of one gather call's one-hot blocks are built
    with a SINGLE DVE tensor_tensor:
        oh[p, (e, j)] = (iota_rep[j] == segmb[p, e])     # stride-0 bcast
    where segmb = segid - (window*512 + sub*128) is precomputed host-side
    per mm entry (DVE instruction overhead ~0.5-1us dwarfs the per-element
    cost, so batching 5-15 builds into one op is ~10x cheaper).
  - For stage A the PSUM window is Xe^T [F x 512 edge-cols]; for stage C
    it is Xv2^T [F x 512 node-cols] which feeds the dense head directly
    as matmul lhsT (no transposes in the dense path; the host supplies
    deg-scaled X^T).
  - Everything is bf16 on the wire (gather rows are 256B packets, the
    AllGather payload halves); PSUM accumulates in f32.
  - SPMD: one program for all 8 cores.  The schedule is static, sized by
    the max token count over cores per (window, chunk/region) cell; each
    cell is one 1024-slot gather call whose runtime count (num_idxs_reg)
    is that max, so padding costs ~3% packets, not 30%.
  - Stage C accumulates region passes into an SBUF-resident Xv2 table
    (24.5KB/partition) so PSUM windows stay short-lived; the dense head
    runs fused per 128-node group right after its window closes.
  - Two AllGathers (edge regions) pipeline behind stage A / stage C
    descriptor generation.

Hardware constraints baked in (v1 + this session's trace/sim analysis):
  - dma_gather idx dtype is int16 -> gather tables <= 32767 rows: X is
    split in 4 chunks of 25000, Xe tables in 2 regions (24576/25600 rows)
  - elem_size_bytes % 256 == 0 -> bf16 F=128 rows (256B) are legal
  - gather output layout: token i -> partition i%128, col-block i//128,
    which is exactly the PE contraction layout
  - SWDGE requires num_idxs_reg == #(idx >= 0) and valid idxs contiguous
    from slot 0 (trailing -1 pads); slots past the count are never
    written, so the boundary block's tail is zero-filled via a small DMA
    (engine ops can't start at arbitrary partitions)
  - PSUM accumulation "zero regions" are 2KB (a whole bank): start/stop
    flags bracket a window's full mm list, and every PSUM tile is padded
    to a full bank so start_tensor_calc can't corrupt a neighbor
  - the Tile sem-assignment pass locks each DMASW semaphore lane to one
    SWDGE queue; lanes rotate over Pool-engine DMA insts in SCHEDULED
    order, so queue_num is assigned AFTER scheduling as lane % 4
  - per-instruction overhead (~0.3-1us) dominates small ops: one-hot
    builds are batched per call, DMA triggers are spread across sync /
    scalar engines
"""

import numpy as np

# ---------------------------------------------------------------- constants
N = 100000
E = 50000
NNZ = 300000
F = 128
NC = 8

NODE_SH = 12500              # nodes per core
NODE_SH_P = 12544            # 98 groups of 128
NGRP = 98
CW = 25                      # 512-node windows (last = 256)

EDGE_SH = 6250               # edges per core
EDGE_SH_P = 6272             # 49 subwindows of 128
AW = 13                      # 512-edge windows (last = 128)
NCH = 4
CHUNK = 25000                # X chunk rows (int16 gather limit)

REG_SPLIT = 3072             # local edge rows [0,3072) -> region 0
REG_ROWS = (3072, 3200)      # padded local rows per region
TBL_ROWS = (NC * 3072, NC * 3200)

T = 1024                     # slots per gather call (8 blocks of 128)

_STATE = {}


# ---------------------------------------------------------------- host side
def _wrap16(idx):
    """[ncall, 1024] int -> [128, ncall*64] int16 SBUF image (idx i at
    partition i%16, col i//16; replicated across the 8 groups of 16)."""
    ncall = idx.shape[0]
    t = idx.reshape(ncall, 64, 16).transpose(0, 2, 1).astype(np.int16)
    t = np.tile(t, (1, 8, 1))                      # [ncall, 128, 64]
    return np.ascontiguousarray(t.transpose(1, 0, 2).reshape(128, ncall * 64))


def _segmb_img(sg, entries, emax):
    """Per-entry biased segid image: [128, ncall*emax] f32 where column
    k*emax+e holds segid(block(e)) - base(e) for call k (never a valid
    one-hot match for pad slots: segid -1 -> negative)."""
    ncall = sg.shape[0]
    img = np.full((128, ncall * emax), -3.0e6, np.float32)
    for k in range(ncall):
        for e, (b, base) in enumerate(entries[k]):
            img[:, k * emax + e] = sg[k, b * 128:(b + 1) * 128] - base
    return np.ascontiguousarray(img)


def _build_stream(owner, cell, gidx, segid, ncells):
    """Pack tokens into per-(core, cell) sorted slot arrays."""
    ga = np.zeros((NC, ncells, T), np.int64)
    sg = np.full((NC, ncells, T), -1.0, np.float32)
    cnt = np.zeros((NC, ncells), np.int64)
    for m in range(NC):
        s = np.nonzero(owner == m)[0]
        o = np.lexsort((segid[s], cell[s]))
        s = s[o]
        cs = cell[s]
        bounds = np.searchsorted(cs, np.arange(ncells + 1))
        for k in range(ncells):
            lo, hi = int(bounds[k]), int(bounds[k + 1])
            n = hi - lo
            if n > T:
                return None
            ga[m, k, :n] = gidx[s[lo:hi]]
            sg[m, k, :n] = segid[s[lo:hi]]
            cnt[m, k] = n
    return ga, sg, cnt


def _cell_entries(sg_all, k, cnt_max, wbase, nsub):
    """Static (block, sub) entry list for one cell, from the union of all
    cores' segids.  Returns None if a segid falls outside the window."""
    entries = []
    nb = -(-int(cnt_max) // 128)
    for b in range(nb):
        vals = sg_all[:, k, b * 128:(b + 1) * 128].ravel()
        vals = vals[vals >= 0].astype(np.int64)
        if len(vals) == 0:
            continue
        for sub in np.unique(vals // 128):
            s = int(sub) - wbase // 128
            if s < 0 or s >= nsub:
                return None
            entries.append((b, s))
    return entries


def _route(vertex, edges):
    eo = edges // EDGE_SH
    le = edges % EDGE_SH
    vo = vertex // NODE_SH
    lv = vertex % NODE_SH
    ch = vertex // CHUNK
    reg = (le >= REG_SPLIT).astype(np.int64)
    trow = np.where(reg == 0, eo * REG_ROWS[0] + le,
                    eo * REG_ROWS[1] + (le - REG_SPLIT))

    rA = _build_stream(eo, (le // 512) * 4 + ch, vertex - ch * CHUNK,
                       le.astype(np.float32), AW * NCH)
    if rA is None:
        return None
    gaA, sgA, cntA = rA
    rC = _build_stream(vo, reg * CW + lv // 512, trow,
                       lv.astype(np.float32), 2 * CW)
    if rC is None:
        return None
    gaC, sgC, cntC = rC

    cntA_max = cntA.max(axis=0)
    cntC_max = cntC.max(axis=0)
    # Slots beyond the static (max-over-cores) count must be NEGATIVE: the
    # SWDGE ucode requires num_idxs_reg == #(idx >= 0) and stops at the
    # last non-negative slot.  Slots in [cnt_m, cnt_max) keep idx 0 (valid
    # row; their segid is -1 so the one-hot zeroes them out).
    for k in range(AW * NCH):
        gaA[:, k, int(cntA_max[k]):] = -1
    for k in range(2 * CW):
        gaC[:, k, int(cntC_max[k]):] = -1

    # Entry lists per call: (block, sub) plus base for segmb biasing.
    entA = []          # per call: [(b, base)]
    mmA = []           # per window: [(cell, entry_idx, block, sub, st, sp)]
    for w in range(AW):
        nsub = 4 if w < AW - 1 else 1
        wlist = []
        for c in range(NCH):
            k = w * 4 + c
            ents = _cell_entries(sgA, k, cntA_max[k], w * 512, nsub)
            if ents is None:
                return None
            entA.append([(b, w * 512 + s * 128) for (b, s) in ents])
            wlist += [(k, e, b, s) for e, (b, s) in enumerate(ents)]
        if set(x[3] for x in wlist) != set(range(nsub)):
            return None
        mmA.append([(k, e, b, s, i == 0, i == len(wlist) - 1)
                    for i, (k, e, b, s) in enumerate(wlist)])

    entC = []
    mmC = []           # per cell k: [(entry, block, sub, st, sp)]
    for r in range(2):
        for w in range(CW):
            nsub = 4 if w < CW - 1 else 2
            k = r * CW + w
            ents = _cell_entries(sgC, k, cntC_max[k], w * 512, nsub)
            if ents is None or not ents:
                return None
            if set(s for (_, s) in ents) != set(range(nsub)):
                return None
            entC.append([(b, w * 512 + s * 128) for (b, s) in ents])
            mmC.append([(i, b, s, i == 0, i == len(ents) - 1)
                        for i, (b, s) in enumerate(ents)])

    emaxA = max(len(x) for x in entA)
    emaxC = max(len(x) for x in entC)
    sig = repr((tuple(cntA_max), tuple(cntC_max), mmA, mmC, emaxA, emaxC))
    return dict(gaA=gaA, sgA=sgA, cntA=cntA_max, mmA=mmA, entA=entA,
                gaC=gaC, sgC=sgC, cntC=cntC_max, mmC=mmC, entC=entC,
                emaxA=emaxA, emaxC=emaxC, sig=sig)


def _numpy_fallback(X, vertex, edges, w_b, w_a, w_c, b_b, b_a, b_c):
    Xe = np.zeros((E, F), np.float32)
    np.add.at(Xe, edges, X[vertex])
    Xv2 = np.zeros((N, F), np.float32)
    np.add.at(Xv2, vertex, Xe[edges])
    deg = np.bincount(vertex, minlength=N).astype(np.float32)[:, None]
    Xv = np.concatenate([deg * X, Xv2], axis=1)
    center = Xv @ w_b + b_b
    aXv = np.abs(Xv)
    return (center.astype(np.float32),
            (center - (aXv @ w_a + b_a)).astype(np.float32),
            (center + (aXv @ w_c + b_c)).astype(np.float32))


# ------------------------------------------------------------- bass program
def _build_program(meta):
    from concourse import bacc, tile
    import concourse.mybir as mybir

    f32 = mybir.dt.float32
    bf16 = mybir.dt.bfloat16
    i16 = mybir.dt.int16
    Alu = mybir.AluOpType
    Abs = mybir.ActivationFunctionType.Abs
    Copy = mybir.ActivationFunctionType.Copy

    cntA, mmA, entA = meta["cntA"], meta["mmA"], meta["entA"]
    cntC, mmC, entC = meta["cntC"], meta["mmC"], meta["entC"]
    emaxA, emaxC = meta["emaxA"], meta["emaxC"]
    emax = max(emaxA, emaxC)

    NQ = 4
    nc = bacc.Bacc(None, target_bir_lowering=False, debug=False,
                   num_devices=NC, num_swdge_queues=NQ)

    xq = [nc.dram_tensor(f"xq{c}", [CHUNK, F], bf16, kind="ExternalInput")
          for c in range(NCH)]
    gaA_d = nc.dram_tensor("gaA", [128, AW * NCH * 64], i16, kind="ExternalInput")
    sgA_d = nc.dram_tensor("sgA", [128, AW * NCH * emaxA], f32, kind="ExternalInput")
    gaC_d = nc.dram_tensor("gaC", [128, 2 * CW * 64], i16, kind="ExternalInput")
    sgC_d = nc.dram_tensor("sgC", [128, 2 * CW * emaxC], f32, kind="ExternalInput")
    xdt_d = nc.dram_tensor("xdt", [128, NODE_SH_P], bf16, kind="ExternalInput")
    wb1_d = nc.dram_tensor("wb1", [F, F], bf16, kind="ExternalInput")
    wb2_d = nc.dram_tensor("wb2", [F, F], bf16, kind="ExternalInput")
    wac1_d = nc.dram_tensor("wac1", [F, 2 * F], bf16, kind="ExternalInput")
    wac2_d = nc.dram_tensor("wac2", [F, 2 * F], bf16, kind="ExternalInput")
    bb_d = nc.dram_tensor("bb", [1, F], bf16, kind="ExternalInput")
    bac_d = nc.dram_tensor("bac", [1, 2 * F], bf16, kind="ExternalInput")
    iota_d = nc.dram_tensor("iota", [128, emax * 128], f32, kind="ExternalInput")
    ident_d = nc.dram_tensor("ident", [128, 128], bf16, kind="ExternalInput")
    out3 = nc.dram_tensor("out3", [NODE_SH_P, 3 * F], bf16, kind="ExternalOutput")

    xe = [nc.dram_tensor(f"xe{r}", [REG_ROWS[r], F], bf16) for r in range(2)]
    xt = [nc.dram_tensor(f"xt{r}", [TBL_ROWS[r], F], bf16) for r in range(2)]

    with tile.TileContext(nc) as tc:
        with (
            tc.tile_pool(name="cp", bufs=1) as cp,
            tc.tile_pool(name="dp", bufs=10) as dp,
            tc.tile_pool(name="ohp", bufs=6) as ohp,
            tc.tile_pool(name="sp", bufs=4) as sp,
            tc.tile_pool(name="psw", bufs=2, space="PSUM") as psw,
            tc.tile_pool(name="psd", bufs=2, space="PSUM") as psd,
        ):
            # ---- constants / preloads
            iota = cp.tile([128, emax * 128], f32, tag="iota")
            nc.sync.dma_start(iota[:], iota_d[:])
            identb = cp.tile([128, 128], bf16, tag="identb")
            nc.sync.dma_start(identb[:], ident_d[:])
            ones1 = cp.tile([1, F], bf16, tag="ones1")
            nc.vector.memset(ones1[:], 1.0)
            wb1s = cp.tile([F, F], bf16, tag="wb1s")
            nc.sync.dma_start(wb1s[:], wb1_d[:])
            wb2s = cp.tile([F, F], bf16, tag="wb2s")
            nc.sync.dma_start(wb2s[:], wb2_d[:])
            wac1s = cp.tile([F, 2 * F], bf16, tag="wac1s")
            nc.sync.dma_start(wac1s[:], wac1_d[:])
            wac2s = cp.tile([F, 2 * F], bf16, tag="wac2s")
            nc.sync.dma_start(wac2s[:], wac2_d[:])
            bbs = cp.tile([1, F], bf16, tag="bbs")
            nc.sync.dma_start(bbs[:], bb_d[:])
            bacs = cp.tile([1, 2 * F], bf16, tag="bacs")
            nc.sync.dma_start(bacs[:], bac_d[:])
            gaA_sb = cp.tile([128, AW * NCH * 64], i16, tag="gaA_sb")
            nc.scalar.dma_start(gaA_sb[:], gaA_d[:])
            sgA_sb = cp.tile([128, AW * NCH * emaxA], f32, tag="sgA_sb")
            nc.scalar.dma_start(sgA_sb[:], sgA_d[:])
            gaC_sb = cp.tile([128, 2 * CW * 64], i16, tag="gaC_sb")
            nc.scalar.dma_start(gaC_sb[:], gaC_d[:])
            sgC_sb = cp.tile([128, 2 * CW * emaxC], f32, tag="sgC_sb")
            nc.scalar.dma_start(sgC_sb[:], sgC_d[:])
            xdt_sb = cp.tile([128, NODE_SH_P], bf16, tag="xdt_sb")
            nc.sync.dma_start(xdt_sb[:], xdt_d[:])
            xv2sb = cp.tile([128, NODE_SH_P], bf16, tag="xv2sb")

            # bias_b broadcast tile (ones-matmul trick).  All psd tiles are
            # padded to a full 2KB PSUM bank: start_tensor_calc marks the
            # whole bank pending-zero, so co-resident tiles would corrupt
            # each other.
            psb = psd.tile([128, 512], f32, tag="pscen")
            nc.tensor.matmul(psb[:, :F], ones1[:], bbs[:], start=True, stop=True)
            bcb = cp.tile([128, F], f32, tag="bcb")
            nc.vector.tensor_copy(bcb[:], psb[:, :F])

            zeros = cp.tile([128, F], bf16, tag="zeros")
            nc.vector.memset(zeros[:], 0.0)

            def tail_memset(dat, cnt):
                # Slots >= the gather's runtime count are never written by
                # the DMA (trailing -1 idxs); zero the boundary block's tail
                # so the one-hot matmuls read finite data.  Engine ops can
                # only start at partition 0/32/64/96, so use a DMA copy.
                pb, pp = cnt // 128, cnt % 128
                if pp:
                    nc.sync.dma_start(dat[pp:128, pb, :], zeros[pp:128, :])

            def build_oh(sg_sb, emax_s, k, nent, tag):
                # One DVE op builds every one-hot block of call k:
                # oh[p, (e, j)] = (iota[j] == segmb[p, k*emax+e])
                oh = ohp.tile([128, emax * 128], bf16, tag=tag)
                segb = sg_sb[:, k * emax_s:k * emax_s + nent]
                nc.vector.tensor_tensor(
                    oh[:, :nent * 128].rearrange("p (e j) -> p e j", e=nent),
                    iota[:, :nent * 128].rearrange("p (e j) -> p e j", e=nent),
                    segb.unsqueeze(-1).broadcast_to([128, nent, 128]),
                    op=Alu.is_equal)
                return oh

            # ---- stage A: Xe^T windows
            for w in range(AW):
                nsub = 4 if w < AW - 1 else 1
                wsz = nsub * 128
                ps = psw.tile([128, 512], f32, tag="psA")
                dats = {}
                ohs = {}
                for c in range(NCH):
                    k = w * 4 + c
                    if cntA[k] == 0:
                        continue
                    dat = dp.tile([128, 8, F], bf16, tag="dA")
                    nc.gpsimd.dma_gather(
                        dat[:], xq[c].ap(), gaA_sb[:, k * 64:(k + 1) * 64],
                        T, int(cntA[k]), F)
                    tail_memset(dat, int(cntA[k]))
                    dats[c] = dat
                    if entA[k]:
                        ohs[c] = build_oh(sgA_sb, emaxA, k, len(entA[k]), "ohA")
                for (k, e, b, s, st, sp_) in mmA[w]:
                    nc.tensor.matmul(ps[:, s * 128:(s + 1) * 128],
                                     dats[k % 4][:, b, :],
                                     ohs[k % 4][:, e * 128:(e + 1) * 128],
                                     start=st, stop=sp_)
                xs = sp.tile([128, 512], bf16, tag="xs")
                nc.scalar.activation(xs[:, :wsz], ps[:, :wsz], Copy)
                for s in range(nsub):
                    S = w * 4 + s
                    pt = psd.tile([128, 1024], bf16, tag="pscen")
                    nc.tensor.transpose(pt[:, :128], xs[:, s * 128:(s + 1) * 128],
                                        identb[:])
                    xo = sp.tile([128, 128], bf16, tag="xo")
                    nc.vector.tensor_copy(xo[:], pt[:, :128])
                    r, row = (0, S * 128) if S < 24 else (1, S * 128 - 3072)
                    nc.scalar.dma_start(xe[r][row:row + 128, :], xo[:])
                if w == 5:
                    nc.gpsimd.collective_compute(
                        "AllGather", Alu.bypass,
                        replica_groups=[list(range(NC))],
                        ins=[xe[0].ap().opt()], outs=[xt[0].ap().opt()])
                if w == AW - 1:
                    nc.gpsimd.collective_compute(
                        "AllGather", Alu.bypass,
                        replica_groups=[list(range(NC))],
                        ins=[xe[1].ap().opt()], outs=[xt[1].ap().opt()])

            # ---- stage C: Xv2^T windows + fused dense head
            for r in range(2):
                for w in range(CW):
                    nsub = 4 if w < CW - 1 else 2
                    wsz = nsub * 128
                    k = r * CW + w
                    ps = psw.tile([128, 512], f32, tag="psC")
                    dat = dp.tile([128, 8, F], bf16, tag="dC")
                    nc.gpsimd.dma_gather(
                        dat[:], xt[r].ap(), gaC_sb[:, k * 64:(k + 1) * 64],
                        T, int(cntC[k]), F)
                    tail_memset(dat, int(cntC[k]))
                    oh = build_oh(sgC_sb, emaxC, k, len(entC[k]), "ohC")
                    for (e, b, s, st, sp_) in mmC[k]:
                        nc.tensor.matmul(ps[:, s * 128:(s + 1) * 128],
                                         dat[:, b, :],
                                         oh[:, e * 128:(e + 1) * 128],
                                         start=st, stop=sp_)
                    sl = xv2sb[:, w * 512:w * 512 + wsz]
                    if r == 0:
                        nc.vector.tensor_copy(sl, ps[:, :wsz])
                        continue
                    nc.vector.tensor_add(sl, sl, ps[:, :wsz])
                    for g in range(w * 4, min(w * 4 + nsub, NGRP)):
                        xd = xdt_sb[:, g * 128:(g + 1) * 128]
                        v2 = xv2sb[:, g * 128:(g + 1) * 128]
                        axd = sp.tile([128, 128], bf16, tag="axd")
                        nc.scalar.activation(axd[:], xd, Abs)
                        av2 = sp.tile([128, 128], bf16, tag="av2")
                        nc.scalar.activation(av2[:], v2, Abs)
                        pc = psd.tile([128, 512], f32, tag="pscen")
                        nc.tensor.matmul(pc[:, :F], xd, wb1s[:], start=True, stop=False)
                        nc.tensor.matmul(pc[:, :F], v2, wb2s[:], start=False, stop=True)
                        pl = psd.tile([128, 512], f32, tag="pslr")
                        nc.tensor.matmul(pl[:, :2 * F], axd[:], wac1s[:],
                                         start=True, stop=False)
                        nc.tensor.matmul(pl[:, :2 * F], av2[:], wac2s[:],
                                         start=False, stop=False)
                        nc.tensor.matmul(pl[:, :2 * F], ones1[:], bacs[:],
                                         start=False, stop=True)
                        ot = sp.tile([128, 3 * F], bf16, tag="ot")
                        nc.vector.tensor_add(ot[:, 0:F], bcb[:], pc[:, :F])
                        nc.vector.tensor_sub(ot[:, F:2 * F], ot[:, 0:F], pl[:, 0:F])
                        nc.vector.tensor_add(ot[:, 2 * F:3 * F], ot[:, 0:F],
                                             pl[:, F:2 * F])
                        rows = min(128, NODE_SH - g * 128)
                        nc.scalar.dma_start(out3[g * 128:g * 128 + rows, :],
                                            ot[:rows, :])

    # SWDGE queue assignment must match the DMASW semaphore lane the Tile
    # sem-assignment pass gave each gather (lanes rotate over Pool-engine
    # DMA insts in SCHEDULED order, which differs from emission order; a
    # lane's semaphore is locked to one queue).  queue = lane % NQ keeps
    # every lane on exactly one queue while spreading descriptor-gen work
    # across all 4 Q7 ucode workers.
    from concourse.tile_sem_assignment import PROC_NAME_TO_IDX
    idx2lane = {PROC_NAME_TO_IDX[f"DMASW{i}"]: i for i in range(8)}
    for insts in tc.ordered_instructions_by_block.values():
        for inst in insts:
            if isinstance(inst, mybir.InstDMAGatherAnt):
                lane = idx2lane.get(getattr(inst, "bass_scheduled_proc", -1))
                if lane is not None:
                    inst.queue_num = lane % NQ

    nc.compile()
    return nc


# ------------------------------------------------------------------- driver
def kernel(X, vertex, edges, X0, n_edges, w_b, w_a, w_c, b_b, b_a, b_c):
    from concourse.bass_utils import run_bass_kernel_spmd
    import ml_dtypes

    bf = ml_dtypes.bfloat16
    X = np.ascontiguousarray(np.asarray(X, dtype=np.float32))
    vertex = np.asarray(vertex).astype(np.int64)
    edges = np.asarray(edges).astype(np.int64)
    w_b = np.asarray(w_b, dtype=np.float32)
    w_a = np.asarray(w_a, dtype=np.float32)
    w_c = np.asarray(w_c, dtype=np.float32)
    b_b = np.asarray(b_b, dtype=np.float32).reshape(1, F)
    b_a = np.asarray(b_a, dtype=np.float32).reshape(1, F)
    b_c = np.asarray(b_c, dtype=np.float32).reshape(1, F)

    meta = _route(vertex, edges)
    if meta is None:
        return _numpy_fallback(X, vertex, edges, w_b, w_a, w_c, b_b, b_a, b_c)

    if _STATE.get("sig") != meta["sig"]:
        _STATE["nc"] = _build_program(meta)
        _STATE["sig"] = meta["sig"]
    nc = _STATE["nc"]

    Xbf = X.astype(bf)
    deg = np.bincount(vertex, minlength=N).astype(np.float32)
    XD = (X * deg[:, None]).astype(np.float32)

    emax = max(meta["emaxA"], meta["emaxC"])
    iota_np = np.ascontiguousarray(
        np.tile(np.arange(128, dtype=np.float32), (128, emax)))
    ident_np = np.ascontiguousarray(np.eye(128, dtype=np.float32).astype(bf))
    wb1 = np.ascontiguousarray(w_b[:F].astype(bf))
    wb2 = np.ascontiguousarray(w_b[F:].astype(bf))
    wac1 = np.ascontiguousarray(
        np.concatenate([w_a[:F], w_c[:F]], axis=1).astype(bf))
    wac2 = np.ascontiguousarray(
        np.concatenate([w_a[F:], w_c[F:]], axis=1).astype(bf))
    bb = np.ascontiguousarray(b_b.astype(bf))
    bac = np.ascontiguousarray(np.concatenate([b_a, b_c], axis=1).astype(bf))

    in_maps = []
    for m in range(NC):
        xdm = np.zeros((128, NODE_SH_P), np.float32)
        xdm[:, :NODE_SH] = XD[m * NODE_SH:(m + 1) * NODE_SH].T
        im = {
            "gaA": _wrap16(meta["gaA"][m]),
            "sgA": _segmb_img(meta["sgA"][m], meta["entA"], meta["emaxA"]),
            "gaC": _wrap16(meta["gaC"][m]),
            "sgC": _segmb_img(meta["sgC"][m], meta["entC"], meta["emaxC"]),
            "xdt": np.ascontiguousarray(xdm.astype(bf)),
            "wb1": wb1, "wb2": wb2, "wac1": wac1, "wac2": wac2,
            "bb": bb, "bac": bac,
            "iota": iota_np, "ident": ident_np,
        }
        for c in range(NCH):
            im[f"xq{c}"] = np.ascontiguousarray(Xbf[c * CHUNK:(c + 1) * CHUNK])
        in_maps.append(im)

    res = run_bass_kernel_spmd(nc, in_maps, list(range(NC)))
    out = np.concatenate(
        [np.asarray(res.results[m]["out3"])[:NODE_SH].astype(np.float32)
         for m in range(NC)], axis=0)
    return (np.ascontiguousarray(out[:, 0:F]),
            np.ascontiguousarray(out[:, F:2 * F]),
            np.ascontiguousarray(out[:, 2 * F:3 * F]))
